# revision 15
# baseline (speedup 1.0000x reference)
"""Conv1d (B=32, C_in=256, L=4096, C_out=512, K=9, stride=1, pad=4) on 8 trn2 cores.

Winograd 3xF(6,3): the K=9 kernel splits into 3 shifted K=3 taps; each uses
F(6,3) (8 points per 6 outputs) => 2.25x fewer PE MACs than direct conv.

Data-parallel over batch: 4 batches/core. Per core, per batch:
  input transform (GpSimd + a DVE slice): V[ci][fam, p, T] = B^T windows of
    x[ci, 6T+3*fam .. +7]  (bf16, strided-column reads from x)
  main matmuls (PE): M_p[Tg(<=128), co(512)] += V_p-slice^T @ U[p,j,ci]
    accumulated over j(3) x ci-chunk(2); 8 points -> 8 PSUM banks
  output transform (DVE): Y_i[Tg, co] = sum_p A^T[i,p] M_p + bias  (bf16)
  DMA Y_i to out[b, T, i, co]; host trims 4098->4096 rows, casts fp32.
"""

import numpy as np

B, C_IN, L = 32, 256, 4096
C_OUT, KW = 512, 9
PAD = 4
N_CORES = 8
B_LOC = B // N_CORES  # 4
P = 128
CI_CH = C_IN // P  # 2
NT = 683           # output tiles of 6 (4098 rows, last 2 trimmed on host)
NTV = 684          # V tiles per family (famA needs T=683 for the j=2 shift)
LPX = 4110         # padded x length: 685*6 (pad 4 left, 10 right)
TG = 128           # T-tiles per matmul group
G_SZ = [128, 128, 128, 128, 128, 43]
NG = len(G_SZ)
DV_T0 = 412        # famB/ci1 input-transform columns >= DV_T0 go to DVE

_cache = {}

_G = np.array([
    [1, 0, 0], [-2/9, -2/9, -2/9], [-2/9, 2/9, -2/9],
    [1/90, 1/45, 2/45], [1/90, -1/45, 2/45],
    [32/45, 16/45, 8/45], [32/45, -16/45, 8/45], [0, 0, 1],
], dtype=np.float64)


def _bf16(a):
    import ml_dtypes
    return np.asarray(a, dtype=np.float32).astype(ml_dtypes.bfloat16)


def _build_program(repeat=1):
    from contextlib import ExitStack

    import concourse.tile as tile
    from concourse import bacc, mybir

    f32 = mybir.dt.float32
    bf16 = mybir.dt.bfloat16
    MULT = mybir.AluOpType.mult
    ADD = mybir.AluOpType.add
    SUB = mybir.AluOpType.subtract

    nc = bacc.Bacc("TRN2", debug=False)
    # x stored phase-major: [b, ci-chunk, 128, phase(6), 685] so every
    # transform window read is a contiguous column slice.
    x_d = nc.dram_tensor("x", [B_LOC, CI_CH, P, LPX], bf16, kind="ExternalInput").ap()
    u_d = nc.dram_tensor("u", [8 * 3 * CI_CH, P, C_OUT], bf16, kind="ExternalInput").ap()
    b_d = nc.dram_tensor("bias", [C_OUT], bf16, kind="ExternalInput").ap()
    o_d = nc.dram_tensor("out", [B_LOC, NT, 6, C_OUT], bf16, kind="ExternalOutput").ap()

    with tile.TileContext(nc) as tc:
        with ExitStack() as ctx:
            persist = ctx.enter_context(tc.tile_pool(name="persist", bufs=1))
            u_sb = persist.tile([P, 8 * 3 * CI_CH * C_OUT], bf16, name="u_sb", tag="u")
            # bias folded into PE: A^T column p=1 is all-ones, so a K=1
            # matmul ones[1,S] x bias[1,co] accumulated into bank 1 adds
            # bias to every output row.
            bias_sb = persist.tile([P, C_OUT], bf16, name="bias_sb", tag="bias")
            ones_sb = persist.tile([P, TG], bf16, name="ones_sb", tag="ones")
            nc.gpsimd.memset(ones_sb[:], 1.0)

            xpool = ctx.enter_context(tc.tile_pool(name="xp", bufs=1))
            vpool = ctx.enter_context(tc.tile_pool(name="vp", bufs=2))
            tpool = ctx.enter_context(tc.tile_pool(name="tp", bufs=2))
            opool = ctx.enter_context(tc.tile_pool(name="op", bufs=2))
            psum_pool = ctx.enter_context(
                tc.tile_pool(name="psum", bufs=8, space="PSUM")
            )

            def new_x(b):
                return [
                    xpool.tile([P, LPX], bf16, name=f"xt{c}", tag=f"xt{c}")
                    for c in range(CI_CH)
                ]

            def new_v(b):
                return [
                    vpool.tile([P, 2 * 8 * NTV], bf16, name=f"v{c}", tag=f"v{c}")
                    for c in range(CI_CH)
                ]

            def emit_x(b, xt):
                NS = 4
                W = LPX // NS  # 1027, last slice +2
                for c in range(CI_CH):
                    for s in range(NS):
                        c0 = s * W
                        c1 = LPX if s == NS - 1 else (s + 1) * W
                        nc.sync.dma_start(
                            out=xt[c][:, c0:c1], in_=x_d[b, c, :, c0:c1]
                        )

            # input transform pass: V_p[T] = B^T d, d_q = x[6T + 3*fam + q],
            # for T in [t0, t1). 28 fused ops (out-of-place chains).
            def emit_itrans(eng, etag, xt, vt, c, fam, t0, t1):
                n = t1 - t0
                x2 = xt[c]

                def d(q):
                    ph, off = (3 * fam + q) % 6, (3 * fam + q) // 6
                    return x2[:, ph * (LPX // 6) + t0 + off : ph * (LPX // 6) + t1 + off]

                def tt(name):
                    t = tpool.tile(
                        [P, NTV], f32, name=name, tag=f"t_{name}_{etag}", bufs=1
                    )
                    return t[:, t0:t1]

                base = fam * 8 * NTV

                def vout(p):
                    return vt[c][:, base + p * NTV + t0 : base + p * NTV + t1]

                stt = eng.scalar_tensor_tensor
                t0_, t1_, t2_, t3_ = tt("t0"), tt("t1"), tt("t2"), tt("t3")
                # a = d2 + d6 - 4.25 d4 ; b = d1 + d5 - 4.25 d3
                stt(t0_, d(4), -4.25, d(2), MULT, ADD)
                stt(t1_, d(6), 1.0, t0_, MULT, ADD)
                stt(t0_, d(3), -4.25, d(1), MULT, ADD)
                stt(t2_, d(5), 1.0, t0_, MULT, ADD)
                stt(vout(1), t1_, 1.0, t2_, MULT, ADD)
                stt(vout(2), t1_, 1.0, t2_, MULT, SUB)
                # c = 0.25 d2 - 1.25 d4 + d6 ; dd = 0.5 d1 - 2.5 d3 + 2 d5
                stt(t0_, d(2), 0.25, d(6), MULT, ADD)
                stt(t1_, d(4), -1.25, t0_, MULT, ADD)
                eng.tensor_scalar_mul(t0_, d(5), 2.0)
                stt(t3_, d(3), -2.5, t0_, MULT, ADD)
                stt(t2_, d(1), 0.5, t3_, MULT, ADD)
                stt(vout(3), t1_, 1.0, t2_, MULT, ADD)
                stt(vout(4), t1_, 1.0, t2_, MULT, SUB)
                # e = 4 d2 - 5 d4 + d6 ; f = 2 d1 - 2.5 d3 + 0.5 d5
                stt(t0_, d(2), 4.0, d(6), MULT, ADD)
                stt(t1_, d(4), -5.0, t0_, MULT, ADD)
                eng.tensor_scalar_mul(t0_, d(5), 0.5)
                stt(t3_, d(3), -2.5, t0_, MULT, ADD)
                stt(t2_, d(1), 2.0, t3_, MULT, ADD)
                stt(vout(5), t1_, 1.0, t2_, MULT, ADD)
                stt(vout(6), t1_, 1.0, t2_, MULT, SUB)
                # v0 = d0 - d6 + 5.25(d4 - d2) ; v7 = d7 - d1 + 5.25(d3 - d5)
                stt(t0_, d(2), -1.0, d(4), MULT, ADD)
                stt(t1_, t0_, 5.25, d(0), MULT, ADD)
                stt(vout(0), d(6), -1.0, t1_, MULT, ADD)
                stt(t0_, d(5), -1.0, d(3), MULT, ADD)
                stt(t1_, t0_, 5.25, d(7), MULT, ADD)
                stt(vout(7), d(1), -1.0, t1_, MULT, ADD)

            def emit_itrans_tt(xt, vt, c, fam, t0, t1):
                # GpSimd variant: Pool engine has no scalar_tensor_tensor,
                # so build from tensor_scalar_mul + tensor_add/sub.
                eng = nc.gpsimd
                x2 = xt[c]

                def d(q):
                    ph, off = (3 * fam + q) % 6, (3 * fam + q) // 6
                    return x2[:, ph * (LPX // 6) + t0 + off : ph * (LPX // 6) + t1 + off]

                def tt(name):
                    t = tpool.tile(
                        [P, NTV], f32, name=name, tag=f"t_{name}_g", bufs=1
                    )
                    return t[:, t0:t1]

                base = fam * 8 * NTV

                def vout(p):
                    return vt[c][:, base + p * NTV + t0 : base + p * NTV + t1]

                t0_, t1_, s0, s1 = tt("t0"), tt("t1"), tt("s0"), tt("s1")
                eng.tensor_scalar_mul(t0_, d(4), -4.25)
                eng.tensor_add(t1_, d(2), d(6))
                eng.tensor_add(s0, t0_, t1_)
                eng.tensor_scalar_mul(t0_, d(3), -4.25)
                eng.tensor_add(t1_, d(1), d(5))
                eng.tensor_add(s1, t0_, t1_)
                eng.tensor_add(vout(1), s0, s1)
                eng.tensor_sub(vout(2), s0, s1)
                eng.tensor_scalar_mul(t0_, d(2), 0.25)
                eng.tensor_add(t1_, t0_, d(6))
                eng.tensor_scalar_mul(t0_, d(4), -1.25)
                eng.tensor_add(s0, t1_, t0_)
                eng.tensor_scalar_mul(t0_, d(1), 0.5)
                eng.tensor_scalar_mul(t1_, d(3), -2.5)
                eng.tensor_add(s1, t0_, t1_)
                eng.tensor_scalar_mul(t0_, d(5), 2.0)
                eng.tensor_add(t1_, s1, t0_)
                eng.tensor_add(vout(3), s0, t1_)
                eng.tensor_sub(vout(4), s0, t1_)
                eng.tensor_scalar_mul(t0_, d(2), 4.0)
                eng.tensor_add(t1_, t0_, d(6))
                eng.tensor_scalar_mul(t0_, d(4), -5.0)
                eng.tensor_add(s0, t1_, t0_)
                eng.tensor_scalar_mul(t0_, d(1), 2.0)
                eng.tensor_scalar_mul(t1_, d(3), -2.5)
                eng.tensor_add(s1, t0_, t1_)
                eng.tensor_scalar_mul(t0_, d(5), 0.5)
                eng.tensor_add(t1_, s1, t0_)
                eng.tensor_add(vout(5), s0, t1_)
                eng.tensor_sub(vout(6), s0, t1_)
                eng.tensor_sub(t0_, d(4), d(2))
                eng.tensor_scalar_mul(t1_, t0_, 5.25)
                eng.tensor_sub(s0, d(0), d(6))
                eng.tensor_add(vout(0), t1_, s0)
                eng.tensor_sub(t0_, d(3), d(5))
                eng.tensor_scalar_mul(t1_, t0_, 5.25)
                eng.tensor_sub(s0, d(7), d(1))
                eng.tensor_add(vout(7), t1_, s0)

            def emit_transforms(b, xt, vt, chunks=1):
                bounds = [NTV * i // chunks for i in range(chunks + 1)]
                for s in range(chunks):
                    t0, t1 = bounds[s], bounds[s + 1]
                    emit_itrans_tt(xt, vt, 0, 0, t0, t1)
                for s in range(chunks):
                    t0, t1 = bounds[s], bounds[s + 1]
                    for fam, c in ((0, 1), (1, 0), (1, 1)):
                        emit_itrans(nc.vector, "v", xt, vt, c, fam, t0, t1)

            def emit_group(b, g, vt):
                S = G_SZ[g]
                t0 = g * TG
                ms = []
                for p in range(8):
                    mp = psum_pool.tile([P, C_OUT], f32, name="mp")
                    n_mm = 7 if p == 1 else 6
                    i = 0
                    for j in range(3):
                        fam = 1 if j == 1 else 0
                        sh = 1 if j == 2 else 0
                        for c in range(CI_CH):
                            vbase = (fam * 8 + p) * NTV + t0 + sh
                            nc.tensor.matmul(
                                mp[:S, :],
                                lhsT=vt[c][:, vbase : vbase + S],
                                rhs=u_sb[
                                    :,
                                    ((p * 3 + j) * CI_CH + c) * C_OUT
                                    : ((p * 3 + j) * CI_CH + c + 1) * C_OUT,
                                ],
                                start=(i == 0),
                                stop=(i == n_mm - 1),
                            )
                            i += 1
                    if p == 1:
                        nc.tensor.matmul(
                            mp[:S, :],
                            lhsT=ones_sb[0:1, :S],
                            rhs=bias_sb[0:1, :],
                            start=False,
                            stop=True,
                        )
                    ms.append(mp)
                return ms

            def emit_otrans(b, g, ms):
                S = G_SZ[g]
                stt = nc.vector.scalar_tensor_tensor

                def tt(name):
                    t = opool.tile(
                        [P, C_OUT], f32, name=name, tag=f"o_{name}", bufs=1
                    )
                    return t[:S]

                def yt(i):
                    t = opool.tile([P, C_OUT], bf16, name=f"y{i}", tag="y", bufs=6)
                    return t[:S]

                m = [ms[p][:S] for p in range(8)]
                ys = [None] * 6
                # HW: each DVE op may read only ONE PSUM operand -> ScalarE
                # (which has its own PSUM port) stages m1/m3/m5 into SBUF;
                # DVE pairs them against m2/m4/m6 from PSUM. Bias already in
                # M1 via the PE ones-matmul, so no bias ops here.
                COPY = mybir.ActivationFunctionType.Copy
                c1, c3, c5 = tt("c1"), tt("c3"), tt("c5")
                nc.scalar.activation(c1, m[1], COPY)
                nc.scalar.activation(c3, m[3], COPY)
                nc.scalar.activation(c5, m[5], COPY)
                # even outputs: s+ = m1+m2, t+ = m3+m4, u+ = m5+m6
                sx, tx, ux = tt("sx"), tt("tx"), tt("ux")
                q1, q2 = tt("q1"), tt("q2")
                stt(sx, m[2], 1.0, c1, MULT, ADD)
                stt(tx, m[4], 1.0, c3, MULT, ADD)
                stt(ux, m[6], 1.0, c5, MULT, ADD)
                stt(q1, m[0], 1.0, sx, MULT, ADD)     # m0 + s+
                stt(q2, ux, 1.0, tx, MULT, ADD)
                ys[0] = yt(0)
                stt(ys[0], q1, 1.0, q2, MULT, ADD)
                for i, (ct, cu) in ((2, (4.0, 0.25)), (4, (16.0, 1 / 16))):
                    stt(q1, tx, ct, sx, MULT, ADD)
                    ys[i] = yt(i)
                    stt(ys[i], ux, cu, q1, MULT, ADD)
                # odd outputs: s- = m1-m2, t- = m3-m4, u- = m5-m6
                stt(sx, m[2], -1.0, c1, MULT, ADD)
                stt(tx, m[4], -1.0, c3, MULT, ADD)
                stt(ux, m[6], -1.0, c5, MULT, ADD)
                for i, (ct, cu) in ((1, (2.0, 0.5)), (3, (8.0, 0.125))):
                    stt(q1, tx, ct, sx, MULT, ADD)
                    ys[i] = yt(i)
                    stt(ys[i], ux, cu, q1, MULT, ADD)
                stt(q1, tx, 32.0, sx, MULT, ADD)
                stt(q2, ux, 1 / 32, q1, MULT, ADD)
                ys[5] = yt(5)
                stt(ys[5], m[7], 1.0, q2, MULT, ADD)
                for i in range(6):
                    nc.sync.dma_start(
                        out=o_d[b, g * TG : g * TG + S, i, :], in_=ys[i]
                    )

            # PE warm-up (HAM ramp) on scratch data
            warm_sb = persist.tile([P, P], f32, name="warm_sb", tag="warm")
            nc.gpsimd.memset(warm_sb[:], 1.0)
            warm_ps = psum_pool.tile([P, C_OUT], f32, name="mp")
            for i in range(12):
                nc.tensor.matmul(
                    warm_ps[:, :P],
                    lhsT=warm_sb[:, :P],
                    rhs=warm_sb[:, :P],
                    start=(i == 0),
                    stop=(i == 11),
                )

            for t in range(8 * 3 * CI_CH):
                nc.sync.dma_start(
                    out=u_sb[:, t * C_OUT : (t + 1) * C_OUT], in_=u_d[t]
                )
            nc.sync.dma_start(
                out=bias_sb[:], in_=b_d.unsqueeze(0).to_broadcast((P, C_OUT))
            )

            def body(first=False):
                xs = [new_x(b) for b in range(B_LOC)]
                vs = [new_v(b) for b in range(B_LOC)]
                emit_x(0, xs[0])
                emit_transforms(0, xs[0], vs[0], chunks=2)
                for b in range(B_LOC):
                    if b + 1 < B_LOC:
                        emit_x(b + 1, xs[b + 1])
                        emit_transforms(b + 1, xs[b + 1], vs[b + 1])
                    for g in range(NG):
                        ms = emit_group(b, g, vs[b])
                        emit_otrans(b, g, ms)

            for r in range(repeat):
                body(first=(r == 0))

    nc.compile()
    return nc


def _get_program(repeat=1):
    key = ("nc", repeat)
    if key not in _cache:
        _cache[key] = _build_program(repeat)
    return _cache[key]


def _host_prep(x, w, bias):
    xp = np.pad(x, ((0, 0), (0, 0), (PAD, LPX - L - PAD)))
    xp = xp.reshape(x.shape[0], CI_CH, P, LPX // 6, 6)
    xp = _bf16(np.ascontiguousarray(xp.transpose(0, 1, 2, 4, 3)).reshape(
        x.shape[0], CI_CH, P, LPX))
    wr = np.asarray(w, np.float64).reshape(C_OUT, C_IN, 3, 3)  # [co, ci, j, k]
    U = np.einsum("pk,ocjk->pjco", _G, wr, optimize=True)  # [p, j, ci, co]
    ut = np.zeros((8 * 3 * CI_CH, P, C_OUT), dtype=np.float32)
    for p in range(8):
        for j in range(3):
            for c in range(CI_CH):
                ut[(p * 3 + j) * CI_CH + c] = U[p, j, c * P : (c + 1) * P, :]
    return xp, _bf16(ut), _bf16(bias)


def _make_in_maps(x, w, bias):
    xp, ut, bs = _host_prep(
        np.asarray(x, np.float32), np.asarray(w, np.float32), bias
    )
    return [
        {"x": np.ascontiguousarray(xp[c * B_LOC : (c + 1) * B_LOC]), "u": ut, "bias": bs}
        for c in range(N_CORES)
    ]


def postprocess(out_arr, nb=B):
    o = np.asarray(out_arr).astype(np.float32).reshape(nb, NT * 6, C_OUT)
    return np.ascontiguousarray(o[:, :L])


def _get_runner():
    if "runner" in _cache:
        return _cache["runner"]

    import jax
    from jax.sharding import Mesh, NamedSharding, PartitionSpec
    from jax.experimental.shard_map import shard_map
    from concourse import mybir
    from concourse.bass2jax import (
        _bass_exec_p,
        install_neuronx_cc_hook,
        partition_id_tensor,
    )

    install_neuronx_cc_hook()
    nc = _get_program()
    partition_name = nc.partition_id_tensor.name if nc.partition_id_tensor else None
    in_names, out_names, out_avals, zero_outs = [], [], [], []
    for alloc in nc.m.functions[0].allocations:
        if not isinstance(alloc, mybir.MemoryLocationSet):
            continue
        name = alloc.memorylocations[0].name
        if alloc.kind == "ExternalInput":
            if name != partition_name:
                in_names.append(name)
        elif alloc.kind == "ExternalOutput":
            shape = tuple(alloc.tensor_shape)
            dtype = mybir.dt.np(alloc.dtype)
            out_names.append(name)
            out_avals.append(jax.core.ShapedArray(shape, dtype))
            zero_outs.append(np.zeros(shape, dtype))
    n_params = len(in_names)
    all_names = in_names + out_names
    if partition_name is not None:
        all_names = all_names + [partition_name]

    def _body(*args):
        extra = [partition_id_tensor()] if partition_name is not None else []
        return tuple(
            _bass_exec_p.bind(
                *(list(args) + extra),
                out_avals=tuple(out_avals),
                in_names=tuple(all_names),
                out_names=tuple(out_names),
                lowering_input_output_aliases=(),
                sim_require_finite=True,
                sim_require_nnan=True,
                nc=nc,
            )
        )

    devices = jax.devices()[:N_CORES]
    mesh = Mesh(np.asarray(devices), ("core",))
    sharding = NamedSharding(mesh, PartitionSpec("core"))
    fn = jax.jit(
        shard_map(
            _body,
            mesh=mesh,
            in_specs=(PartitionSpec("core"),) * (n_params + len(out_names)),
            out_specs=(PartitionSpec("core"),) * len(out_names),
            check_rep=False,
        )
    )
    zeros_dev = [
        jax.device_put(np.concatenate([z] * N_CORES, axis=0), sharding)
        for z in zero_outs
    ]
    _cache["runner"] = (fn, in_names, out_names, zeros_dev, sharding)
    return _cache["runner"]


def kernel(**inputs):
    x = np.asarray(inputs["x"], dtype=np.float32)
    w = np.asarray(inputs["weight"], dtype=np.float32)
    bias = np.asarray(inputs["bias"], dtype=np.float32)

    try:
        import jax

        fn, in_names, out_names, zeros_dev, sharding = _get_runner()
        xp, ut, bs = _host_prep(x, w, bias)
        glob = {
            "x": xp,
            "u": np.concatenate([ut] * N_CORES, axis=0),
            "bias": np.concatenate([bs] * N_CORES, axis=0),
        }
        dev_in = [jax.device_put(glob[nm], sharding) for nm in in_names]
        r = fn(*dev_in, *zeros_dev)
        return postprocess(r[out_names.index("out")])
    except Exception:
        from concourse.bass_utils import run_bass_kernel_spmd

        nc = _get_program()
        res = run_bass_kernel_spmd(
            nc, _make_in_maps(x, w, bias), list(range(N_CORES))
        )
        return postprocess(
            np.concatenate([res.results[c]["out"] for c in range(N_CORES)], axis=0)
        )


# revision 16
# speedup vs baseline: 1.7978x; 1.7978x over previous
"""Conv1d (B=32, C_in=256, L=4096, C_out=512, K=9, stride=1, pad=4) on 8 trn2 cores.

Data-parallel over batch: 4 batches per core; weights/bias broadcast.
Per core: out[b, t, co] = sum_{ci,k} x_pad[b, ci, t+k] * w[co, ci, k] + bias[co]
computed as 18 PSUM-accumulated matmuls per 128-position output tile:
  stationary lhsT = x_pad[ci(128), t(128)]  (slid by k)
  moving    rhs  = w_k[ci(128), co(512)]    (host-pre-transposed to [K, C_in, C_out])
PSUM tile [t(128), co(512)] -> +bias on DVE -> DMA to (B, T, C_out) output.
"""

import numpy as np

B, C_IN, L = 32, 256, 4096
C_OUT, KW = 512, 9
PAD = 4
N_CORES = 8
B_LOC = B // N_CORES  # 4
P = 128
CI_CHUNKS = C_IN // P  # 2
T_TILE = 128
LP = L + 2 * PAD  # 4104
N_TT = L // T_TILE  # 32

# matmul input dtype mode: "f32r" (full-rate), "f32" (exact, 4x slower)
MM_MODE = "f32r"

_cache = {}


def _build_program(repeat=1):
    from contextlib import ExitStack

    import concourse.tile as tile
    from concourse import bacc, mybir

    f32 = mybir.dt.float32
    mm_dt = mybir.dt.float32r if MM_MODE == "f32r" else mybir.dt.float32

    nc = bacc.Bacc("TRN2", debug=False)
    x_d = nc.dram_tensor("x", [B_LOC, C_IN, LP], mm_dt, kind="ExternalInput").ap()
    w_d = nc.dram_tensor("w", [KW, C_IN, C_OUT], mm_dt, kind="ExternalInput").ap()
    b_d = nc.dram_tensor("bias", [C_OUT], f32, kind="ExternalInput").ap()
    o_d = nc.dram_tensor("out", [B_LOC, L, C_OUT], f32, kind="ExternalOutput").ap()

    with tile.TileContext(nc) as tc:
        with ExitStack() as ctx:
            persist = ctx.enter_context(tc.tile_pool(name="persist", bufs=1))
            wt = persist.tile(
                [P, KW * CI_CHUNKS * C_OUT], mm_dt, name="wt", tag="wt"
            )
            bias_sb = persist.tile([P, C_OUT], f32, name="bias_sb", tag="bias")
            xps = [
                persist.tile([P, CI_CHUNKS * LP], mm_dt, name=f"xp{i}", tag=f"xp{i}")
                for i in range(2)
            ]

            psum_pool = ctx.enter_context(
                tc.tile_pool(name="psum", bufs=8, space="PSUM")
            )
            out_pool = ctx.enter_context(tc.tile_pool(name="outs", bufs=6))

            NS = 8  # x DMA slices per (batch, ci-chunk): finer deps, earlier start
            SW = LP // NS  # 513
            assert SW * NS == LP

            def emit_w(k):
                # wt column block (k*2+c) holds w[k, c*128:(c+1)*128, :].
                for c in range(CI_CHUNKS):
                    j = (k * CI_CHUNKS + c) * C_OUT
                    nc.sync.dma_start(
                        out=wt[:, j : j + C_OUT], in_=w_d[k, c * P : (c + 1) * P, :]
                    )

            def emit_x(b, slices=range(NS)):
                xp = xps[b % 2]
                for s in slices:
                    for c in range(CI_CHUNKS):
                        nc.sync.dma_start(
                            out=xp[:, c * LP + s * SW : c * LP + (s + 1) * SW],
                            in_=x_d[b, c * P : (c + 1) * P, s * SW : (s + 1) * SW],
                        )

            # Warm-up matmuls on scratch data: PE ramps to full clock (HAM /
            # p-state) during the initial weight/x DMA wait instead of running
            # the first real groups cold. f32 dtype (memset can't produce
            # fp32r); results land in a rotating psum bank, never read.
            NWARM = 12
            if NWARM:
                warm_sb = persist.tile([P, C_OUT], f32, name="warm_sb", tag="warm")
                nc.gpsimd.memset(warm_sb[:], 1.0)
                warm_ps = psum_pool.tile([P, C_OUT], f32, name="ps")
                for i in range(NWARM):
                    nc.tensor.matmul(
                        warm_ps[:, :P],
                        lhsT=warm_sb[:, :P],
                        rhs=warm_sb[:, :P],
                        start=(i == 0),
                        stop=(i == NWARM - 1),
                    )

            # Emission order shapes DMA priority: first-needed data first —
            # k=0 weights, x slice 0, remaining weights, remaining x slices.
            emit_w(0)
            emit_x(0, slices=[0])
            for k in range(1, KW):
                emit_w(k)
            nc.sync.dma_start(
                out=bias_sb[:], in_=b_d.unsqueeze(0).to_broadcast((P, C_OUT))
            )
            emit_x(0, slices=range(1, NS))

            def body(first=False):
                for b in range(B_LOC):
                    if not (first and b == 0):
                        emit_x(b)
                    xp = xps[b % 2]
                    for ti in range(N_TT):
                        t0 = ti * T_TILE
                        ps = psum_pool.tile([P, C_OUT], f32, name="ps")
                        n_mm = KW * CI_CHUNKS
                        i = 0
                        for k in range(KW):
                            for c in range(CI_CHUNKS):
                                j = (k * CI_CHUNKS + c) * C_OUT
                                nc.tensor.matmul(
                                    ps[:],
                                    lhsT=xp[
                                        :, c * LP + t0 + k : c * LP + t0 + k + T_TILE
                                    ],
                                    rhs=wt[:, j : j + C_OUT],
                                    start=(i == 0),
                                    stop=(i == n_mm - 1),
                                )
                                i += 1
                        ob = out_pool.tile([P, C_OUT], f32, name="ob")
                        nc.vector.tensor_add(ob[:], ps[:], bias_sb[:])
                        nc.sync.dma_start(
                            out=o_d[b, t0 : t0 + T_TILE, :], in_=ob[:]
                        )

            for r in range(repeat):
                body(first=(r == 0))

    nc.compile()
    return nc


def _get_program(repeat=1):
    key = ("nc", repeat)
    if key not in _cache:
        _cache[key] = _build_program(repeat)
    return _cache[key]


def _make_in_maps(x, w, bias):
    wt = np.ascontiguousarray(np.transpose(w, (2, 1, 0)))  # (K, C_in, C_out)
    xp = np.pad(x, ((0, 0), (0, 0), (PAD, PAD)))  # (B, C_in, L+2*PAD)
    return [
        {
            "x": np.ascontiguousarray(xp[c * B_LOC : (c + 1) * B_LOC]),
            "w": wt,
            "bias": bias,
        }
        for c in range(N_CORES)
    ]


def _get_runner():
    """Cached SPMD runner: same bass2jax/PJRT execution path that
    run_bass_kernel_spmd uses under axon, but the jitted executable and the
    (constant) zero output operands are built once and reused per call."""
    if "runner" in _cache:
        return _cache["runner"]

    import jax
    from jax.sharding import Mesh, NamedSharding, PartitionSpec
    from jax.experimental.shard_map import shard_map
    from concourse import mybir
    from concourse.bass2jax import (
        _bass_exec_p,
        install_neuronx_cc_hook,
        partition_id_tensor,
    )

    install_neuronx_cc_hook()
    nc = _get_program()
    partition_name = nc.partition_id_tensor.name if nc.partition_id_tensor else None
    in_names, out_names, out_avals, zero_outs = [], [], [], []
    for alloc in nc.m.functions[0].allocations:
        if not isinstance(alloc, mybir.MemoryLocationSet):
            continue
        name = alloc.memorylocations[0].name
        if alloc.kind == "ExternalInput":
            if name != partition_name:
                in_names.append(name)
        elif alloc.kind == "ExternalOutput":
            shape = tuple(alloc.tensor_shape)
            dtype = mybir.dt.np(alloc.dtype)
            out_names.append(name)
            out_avals.append(jax.core.ShapedArray(shape, dtype))
            zero_outs.append(np.zeros(shape, dtype))
    n_params = len(in_names)
    all_names = in_names + out_names
    if partition_name is not None:
        all_names = all_names + [partition_name]

    def _body(*args):
        extra = [partition_id_tensor()] if partition_name is not None else []
        return tuple(
            _bass_exec_p.bind(
                *(list(args) + extra),
                out_avals=tuple(out_avals),
                in_names=tuple(all_names),
                out_names=tuple(out_names),
                lowering_input_output_aliases=(),
                sim_require_finite=True,
                sim_require_nnan=True,
                nc=nc,
            )
        )

    devices = jax.devices()[:N_CORES]
    mesh = Mesh(np.asarray(devices), ("core",))
    sharding = NamedSharding(mesh, PartitionSpec("core"))
    fn = jax.jit(
        shard_map(
            _body,
            mesh=mesh,
            in_specs=(PartitionSpec("core"),) * (n_params + len(out_names)),
            out_specs=(PartitionSpec("core"),) * len(out_names),
            check_rep=False,
        )
    )
    # Zero "output" operands: required custom-call inputs; the kernel writes
    # every output element, so these can be device-resident constants.
    zeros_dev = [
        jax.device_put(np.concatenate([z] * N_CORES, axis=0), sharding)
        for z in zero_outs
    ]
    _cache["runner"] = (fn, in_names, out_names, zeros_dev, sharding)
    return _cache["runner"]


def kernel(**inputs):
    x = np.asarray(inputs["x"], dtype=np.float32)
    w = np.asarray(inputs["weight"], dtype=np.float32)
    bias = np.asarray(inputs["bias"], dtype=np.float32)

    try:
        import jax

        fn, in_names, out_names, zeros_dev, sharding = _get_runner()
        # Global (concat-across-cores) operands; shard c along axis 0 is core
        # c's slice: x -> batches 4c..4c+3 (padded), w/bias -> replicated.
        wt = np.ascontiguousarray(np.transpose(w, (2, 1, 0)))  # (K, C_in, C_out)
        glob = {
            "x": np.pad(x, ((0, 0), (0, 0), (PAD, PAD))),
            "w": np.concatenate([wt] * N_CORES, axis=0),
            "bias": np.concatenate([bias] * N_CORES, axis=0),
        }
        dev_in = [jax.device_put(glob[nm], sharding) for nm in in_names]
        r = fn(*dev_in, *zeros_dev)
        out = np.asarray(r[out_names.index("out")])
        return out.reshape(B, L, C_OUT)
    except Exception:
        # Fallback: the stock SPMD runner (same program, per-core in_maps).
        from concourse.bass_utils import run_bass_kernel_spmd

        nc = _get_program()
        res = run_bass_kernel_spmd(
            nc, _make_in_maps(x, w, bias), list(range(N_CORES))
        )
        return np.concatenate(
            [res.results[c]["out"] for c in range(N_CORES)], axis=0
        )

